# revision 1
# baseline (speedup 1.0000x reference)
"""F1-score (macro) kernel for Trainium2, 8 NeuronCores.

Pipeline per core (data-parallel over rows):
  - stream y_pred tiles [128p, TK, 128c] (row = base + p*TK + k)
  - rowmax via DVE tensor_reduce (X axis)
  - one-hot(pred) = (y_pred >= rowmax) per chunk  -> bf16   (tensor_scalar)
  - one-hot(true) = (iota == y_true) per chunk    -> bf16   (tensor_scalar)
  - confusion matrix accumulated on PE: cm += oh_true.T @ oh_pred (PSUM fp32)
Host: sum the 8 local cms, compute macro-F1 (tiny [128,128] epilogue).
"""

import sys

if "/opt/trn_rl_repo" not in sys.path:
    sys.path.insert(0, "/opt/trn_rl_repo")

import numpy as np

import concourse.bacc as bacc
import concourse.mybir as mybir
import concourse.tile as tile
from concourse import bass_utils

C = 128
N = 1_000_000
NCORES = 8
R = N // NCORES          # 125000 rows per core
TK = 16                  # chunks (of 128 rows) per tile
TR = 128 * TK            # 2048 rows per tile
NT = R // TR             # 61 full tiles
TAIL = R - NT * TR       # 72 rows
EPS = 1e-12

# Engine assignment per chunk index (tunable): True -> gpsimd, False -> vector
GE_ON_GS = [True] * 16   # one-hot(pred) is_ge chunks
EQ_ON_GS = [False] * 16  # one-hot(true) is_equal chunks

_CACHE = {}


def _build():
    f32 = mybir.dt.float32
    bf16 = mybir.dt.bfloat16

    nc = bacc.Bacc("TRN2", target_bir_lowering=False, debug=False,
                   num_devices=NCORES)
    yp = nc.dram_tensor("yp", [R, C], f32, kind="ExternalInput")
    yt = nc.dram_tensor("yt", [R], f32, kind="ExternalInput")
    cm = nc.dram_tensor("cm", [C, C], f32, kind="ExternalOutput")

    with tile.TileContext(nc) as tc:
        with (
            tc.tile_pool(name="const", bufs=1) as cpool,
            tc.tile_pool(name="xin", bufs=4) as xpool,
            tc.tile_pool(name="oh", bufs=3) as ohpool,
            tc.tile_pool(name="small", bufs=4) as spool,
            tc.tile_pool(name="psum", bufs=1, space="PSUM") as psum,
        ):
            # constants: iota 0..127 (bf16) replicated per partition
            iota_i = cpool.tile([128, C], mybir.dt.int32)
            nc.gpsimd.iota(iota_i[:], pattern=[[1, C]], base=0,
                           channel_multiplier=0)
            iota_bf = cpool.tile([128, C], bf16)
            nc.vector.tensor_copy(iota_bf[:], iota_i[:])

            # all y_true for the full tiles in one DMA:
            # t_all[p, n, k] = yt[n*TR + p*TK + k]
            t_all = cpool.tile([128, NT, TK], f32)
            nc.sync.dma_start(
                t_all[:],
                yt.ap()[0 : NT * TR].rearrange("(n p k) -> p n k", p=128, k=TK),
            )

            acc = psum.tile([C, C], f32)

            for i in range(NT):
                x = xpool.tile([128, TK, C], f32)
                nc.sync.dma_start(
                    x[:],
                    yp.ap()[i * TR : (i + 1) * TR, :].rearrange(
                        "(p k) c -> p k c", k=TK
                    ),
                )
                rmax = spool.tile([128, TK], f32)
                nc.vector.tensor_reduce(
                    rmax[:], x[:], axis=mybir.AxisListType.X,
                    op=mybir.AluOpType.max,
                )
                ohp = ohpool.tile([128, TK, C], bf16, tag="ohp")
                oht = ohpool.tile([128, TK, C], bf16, tag="oht")
                for k in range(TK):
                    eng = nc.gpsimd if GE_ON_GS[k] else nc.vector
                    eng.tensor_scalar(
                        ohp[:, k, :], x[:, k, :], rmax[:, k : k + 1], None,
                        op0=mybir.AluOpType.is_ge,
                    )
                    eng = nc.gpsimd if EQ_ON_GS[k] else nc.vector
                    eng.tensor_scalar(
                        oht[:, k, :], iota_bf[:], t_all[:, i, k : k + 1], None,
                        op0=mybir.AluOpType.is_equal,
                    )
                for k in range(TK):
                    nc.tensor.matmul(
                        acc[:], oht[:, k, :], ohp[:, k, :],
                        start=(i == 0 and k == 0), stop=False,
                    )

            # tail rows (72)
            xt = xpool.tile([TAIL, 1, C], f32, tag="xtail")
            nc.sync.dma_start(
                xt[:],
                yp.ap()[NT * TR : R, :].rearrange("(p k) c -> p k c", k=1),
            )
            tt = spool.tile([TAIL, 1], f32, tag="ttail")
            nc.sync.dma_start(
                tt[:], yt.ap()[NT * TR : R].rearrange("(p k) -> p k", k=1)
            )
            rmax_t = spool.tile([TAIL, 1], f32, tag="rmaxtail")
            nc.vector.tensor_reduce(
                rmax_t[:], xt[:], axis=mybir.AxisListType.X,
                op=mybir.AluOpType.max,
            )
            ohp_t = ohpool.tile([TAIL, C], bf16, tag="ohptail")
            oht_t = ohpool.tile([TAIL, C], bf16, tag="ohttail")
            nc.vector.tensor_scalar(
                ohp_t[:], xt[:, 0, :], rmax_t[:], None,
                op0=mybir.AluOpType.is_ge,
            )
            nc.vector.tensor_scalar(
                oht_t[:], iota_bf[:TAIL, :], tt[:], None,
                op0=mybir.AluOpType.is_equal,
            )
            nc.tensor.matmul(
                acc[:], oht_t[:], ohp_t[:], start=False, stop=True,
            )

            out_sb = spool.tile([C, C], f32, tag="out")
            nc.scalar.copy(out_sb[:], acc[:])
            nc.sync.dma_start(cm.ap()[:], out_sb[:])

    nc.compile()
    return nc


def _get_nc():
    if "nc" not in _CACHE:
        _CACHE["nc"] = _build()
    return _CACHE["nc"]


def _run(y_pred, y_true, trace=False):
    nc = _get_nc()
    y_pred = np.ascontiguousarray(np.asarray(y_pred, dtype=np.float32))
    yt_f = np.asarray(y_true).astype(np.float32)
    in_maps = [
        {
            "yp": y_pred[c * R : (c + 1) * R],
            "yt": np.ascontiguousarray(yt_f[c * R : (c + 1) * R]),
        }
        for c in range(NCORES)
    ]
    res = bass_utils.run_bass_kernel_spmd(
        nc, in_maps, core_ids=list(range(NCORES)), trace=trace
    )
    cm = np.zeros((C, C), dtype=np.float64)
    for r in res.results:
        cm += r["cm"].astype(np.float64)
    diag = np.diagonal(cm)
    precision = diag / (cm.sum(axis=1) + EPS)
    recall = diag / (cm.sum(axis=0) + EPS)
    f1 = 2.0 * precision * recall / (precision + recall + EPS)
    return np.float32(f1.mean()), res


def kernel(y_pred, y_true):
    out, _ = _run(y_pred, y_true, trace=False)
    return out


# revision 2
# speedup vs baseline: 6.6046x; 6.6046x over previous
"""F1-score (macro) kernel for Trainium2, 8 NeuronCores.

Per core (data-parallel over rows), per tile of 2048 rows ([128p, 16k, 128c],
row = base + p*16 + k):
  - DVE:  rowmax via tensor_reduce (X axis)
  - ACT:  anti-one-hot(pred) = sign(rowmax - x) in {0,1}  (15 chunks)
  - GS :  1 anti chunk via is_lt, 1 one-hot(true) chunk via is_equal
  - DVE:  one-hot(true) = (iota == y_true) via one broadcast TT (15 chunks)
  - PE :  cm_dev += one_hot_trueT @ anti  (bf16 matmuls, fp32 PSUM)
Host: cm = support[t] - sum_cores(cm_dev);  macro-F1 epilogue on [128,128].
All comparisons in exact fp32 -> bit-exact confusion matrix.
"""

import sys
import time

if "/opt/trn_rl_repo" not in sys.path:
    sys.path.insert(0, "/opt/trn_rl_repo")

import numpy as np

import concourse.bacc as bacc
import concourse.mybir as mybir
import concourse.tile as tile
from concourse import bass_utils

C = 128
N = 1_000_000
NCORES = 8
R = N // NCORES          # 125000 rows per core
TK = 16                  # chunks (of 128 rows) per tile
TR = 128 * TK            # 2048 rows per tile
NT = R // TR             # 61 full tiles
TAIL = R - NT * TR       # 72 rows
EPS = 1e-12

NGS_P = 1                # trailing anti chunks on GpSimd (rest on ACT)
NGS_T = 1                # trailing one-hot(true) chunks on GpSimd (rest DVE)

_CACHE = {}


def _build():
    f32 = mybir.dt.float32
    bf16 = mybir.dt.bfloat16
    Alu = mybir.AluOpType
    Act = mybir.ActivationFunctionType

    nc = bacc.Bacc("TRN2", target_bir_lowering=False, debug=False,
                   num_devices=NCORES)
    yp = nc.dram_tensor("yp", [R, C], f32, kind="ExternalInput")
    yt = nc.dram_tensor("yt", [R], f32, kind="ExternalInput")
    cm = nc.dram_tensor("cm", [C, C], f32, kind="ExternalOutput")

    with tile.TileContext(nc) as tc:
        with (
            tc.tile_pool(name="const", bufs=1) as cpool,
            tc.tile_pool(name="xin", bufs=4) as xpool,
            tc.tile_pool(name="oh", bufs=3) as ohpool,
            tc.tile_pool(name="small", bufs=4) as spool,
            tc.tile_pool(name="psum", bufs=1, space="PSUM") as psum,
        ):
            iota_i = cpool.tile([128, C], mybir.dt.int32)
            nc.gpsimd.iota(iota_i[:], pattern=[[1, C]], base=0,
                           channel_multiplier=0)
            iota_bf = cpool.tile([128, C], bf16)
            nc.vector.tensor_copy(iota_bf[:], iota_i[:])
            iota_rep = cpool.tile([128, TK, C], bf16)
            nc.vector.tensor_copy(
                iota_rep[:], iota_bf[:, None, :].broadcast_to([128, TK, C])
            )

            # all y_true for the full tiles in one DMA:
            # t_all[p, n, k] = yt[n*TR + p*TK + k]
            t_all = cpool.tile([128, NT, TK], f32)
            nc.sync.dma_start(
                t_all[:],
                yt.ap()[0 : NT * TR].rearrange("(n p k) -> p n k", p=128, k=TK),
            )

            acc = psum.tile([C, C], f32)

            for i in range(NT):
                x = xpool.tile([128, TK, C], f32)
                nc.sync.dma_start(
                    x[:],
                    yp.ap()[i * TR : (i + 1) * TR, :].rearrange(
                        "(p k) c -> p k c", k=TK
                    ),
                )
                rmax = spool.tile([128, TK], f32)
                nc.vector.tensor_reduce(
                    rmax[:], x[:], axis=mybir.AxisListType.X, op=Alu.max
                )
                anti = ohpool.tile([128, TK, C], bf16, tag="anti")
                oht = ohpool.tile([128, TK, C], bf16, tag="oht")
                for k in range(TK - NGS_P):
                    nc.scalar.activation(
                        anti[:, k, :], x[:, k, :], Act.Sign,
                        bias=rmax[:, k : k + 1], scale=-1.0,
                    )
                for k in range(TK - NGS_P, TK):
                    nc.gpsimd.tensor_scalar(
                        anti[:, k, :], x[:, k, :], rmax[:, k : k + 1], None,
                        op0=Alu.is_lt,
                    )
                nt_dve = TK - NGS_T
                nc.vector.tensor_tensor(
                    oht[:, 0:nt_dve, :], iota_rep[:, 0:nt_dve, :],
                    t_all[:, i, 0:nt_dve, None].broadcast_to([128, nt_dve, C]),
                    op=Alu.is_equal,
                )
                for k in range(nt_dve, TK):
                    nc.gpsimd.tensor_scalar(
                        oht[:, k, :], iota_bf[:], t_all[:, i, k : k + 1],
                        None, op0=Alu.is_equal,
                    )
                for k in range(TK):
                    nc.tensor.matmul(
                        acc[:], oht[:, k, :], anti[:, k, :],
                        start=(i == 0 and k == 0), stop=False,
                    )

            # tail rows (72), all on DVE
            xt = xpool.tile([TAIL, 1, C], f32, tag="xtail")
            nc.sync.dma_start(
                xt[:],
                yp.ap()[NT * TR : R, :].rearrange("(p k) c -> p k c", k=1),
            )
            tt = spool.tile([TAIL, 1], f32, tag="ttail")
            nc.sync.dma_start(
                tt[:], yt.ap()[NT * TR : R].rearrange("(p k) -> p k", k=1)
            )
            rmax_t = spool.tile([TAIL, 1], f32, tag="rmaxtail")
            nc.vector.tensor_reduce(
                rmax_t[:], xt[:], axis=mybir.AxisListType.X, op=Alu.max
            )
            anti_t = ohpool.tile([TAIL, C], bf16, tag="antitail")
            oht_t = ohpool.tile([TAIL, C], bf16, tag="ohttail")
            nc.vector.tensor_scalar(
                anti_t[:], xt[:, 0, :], rmax_t[:], None, op0=Alu.is_lt
            )
            nc.vector.tensor_scalar(
                oht_t[:], iota_bf[:TAIL, :], tt[:], None, op0=Alu.is_equal
            )
            nc.tensor.matmul(
                acc[:], oht_t[:], anti_t[:], start=False, stop=True
            )

            out_sb = spool.tile([C, C], f32, tag="out")
            nc.scalar.copy(out_sb[:], acc[:])
            nc.sync.dma_start(cm.ap()[:], out_sb[:])

    nc.compile()
    return nc


def _get_nc():
    if "nc" not in _CACHE:
        _CACHE["nc"] = _build()
    return _CACHE["nc"]


def _run(y_pred, y_true, trace=False):
    nc = _get_nc()
    y_pred = np.ascontiguousarray(np.asarray(y_pred, dtype=np.float32))
    yt_i = np.asarray(y_true).astype(np.int64)
    yt_f = yt_i.astype(np.float32)
    in_maps = [
        {
            "yp": y_pred[c * R : (c + 1) * R],
            "yt": np.ascontiguousarray(yt_f[c * R : (c + 1) * R]),
        }
        for c in range(NCORES)
    ]
    res = None
    for attempt in range(3):
        try:
            res = bass_utils.run_bass_kernel_spmd(
                nc, in_maps, core_ids=list(range(NCORES)), trace=trace
            )
            break
        except Exception:
            if attempt == 2:
                raise
            time.sleep(2.0)
    cm_dev = np.zeros((C, C), dtype=np.float64)
    for r in res.results:
        cm_dev += r["cm"].astype(np.float64)
    support = np.bincount(yt_i, minlength=C).astype(np.float64)
    cm = support[:, None] - cm_dev
    diag = np.diagonal(cm)
    precision = diag / (cm.sum(axis=1) + EPS)
    recall = diag / (cm.sum(axis=0) + EPS)
    f1 = 2.0 * precision * recall / (precision + recall + EPS)
    return np.float32(f1.mean()), res


def kernel(y_pred, y_true):
    out, _ = _run(y_pred, y_true, trace=False)
    return out
